# revision 24
# baseline (speedup 1.0000x reference)
"""Chamfer distance kernel for 8 trn2 NeuronCores.

Sharding: data-parallel over batch B=4 (2 cores per batch element), with the
N=8192 predicted-point axis split in half across the core pair. Each core
computes, for its (batch, n-half):
  - d2[n, m] squared-distance tiles directly on the TensorEngine via a K=16
    matmul that folds the whole expression |p|^2 + |t|^2 - 2 p.t into one
    contraction: conceptual rows [-2*px, -2*py, -2*pz, 1, |p|^2] x
    [tx, ty, tz, |t|^2, 1], with every fp32 row split into an fp16 hi/lo pair
    (fp16 matmuls stream 1 col/cycle vs 4 for fp32; the hi/lo split keeps
    ~22 mantissa bits so d2 stays fp32-accurate). PSUM accumulates in fp32.
  - ScalarE converts PSUM fp32 -> SBUF fp16.
  - VectorE computes running elementwise min over n-tiles (-> per-m partial
    mins, [128, 8192], partition p holds min over n = i*128+p) and per-n mins
    over m (free-dim halving tree to 256; the last fold of each tile lands in
    a [128, 4, 256] staging tile so ONE 1x tensor_reduce covers 4 tiles).
Host: final partition-axis min, cross-core min, sqrt, means (fp64), scalar out.

DVE is the bottleneck (gapless at ~9.05us/tile = its busy time; ACT 80%, PE
36%, Pool unusable: walrus rejects TensorTensor on the GPSIMD engine and its
TensorCopy executes incorrectly via the PJRT path). Per-tile DVE floor =
nacc 8192 outs + tree ~8064 outs at 2 elem/cycle (2x_1p) ~= 8.9us; the tuning
left is edges only: tiles 0-3 process per-PSUM-group (DVE is ACT-starved
until ~tile 3), tree L1 is emitted before the nacc (fewer deps), and the last
tile's nacc is chunked so its output DMA overlaps remaining DVE work.
"""

import numpy as np

B = 4
N = 8192
M = 8192
NCORES = 8
NSH = N // 2          # predicted points per core
NT = NSH // 128       # 32 n-tiles per core
KDIM = 16             # fp16 hi/lo split rows (4 per coord + pn pair + tn pair)
MBLK = 512            # matmul free dim (one PSUM bank)
GBLK = 2048           # PSUM group (4 banks) converted per ACT op

_CACHE = {}


def _build_bass():
    from contextlib import ExitStack

    import concourse.bacc as bacc
    import concourse.mybir as mybir
    import concourse.tile as tile

    dt = mybir.dt
    amin = mybir.AluOpType.min
    X = mybir.AxisListType.X

    nc = bacc.Bacc(
        "TRN2",
        target_bir_lowering=False,
        debug=False,
        num_devices=NCORES,
    )
    a_dram = nc.declare_dram_parameter("a", [KDIM, NSH], dt.float16, isOutput=False)
    b_dram = nc.declare_dram_parameter("b", [KDIM, M], dt.float16, isOutput=False)
    out_macc = nc.declare_dram_parameter("out_macc", [128, NT], dt.float32, isOutput=True)
    out_nacc = nc.declare_dram_parameter("out_nacc", [128, M], dt.float16, isOutput=True)

    with ExitStack() as ctx:
        tc = ctx.enter_context(tile.TileContext(nc))
        const_pool = ctx.enter_context(tc.tile_pool(name="const", bufs=1))
        psum_pool = ctx.enter_context(tc.tile_pool(name="psum", bufs=2, space="PSUM"))
        c_pool = ctx.enter_context(tc.tile_pool(name="c", bufs=2))
        nacc_pool = ctx.enter_context(tc.tile_pool(name="nacc", bufs=2))
        macc_pool = ctx.enter_context(tc.tile_pool(name="macc", bufs=2))
        t4_pool = ctx.enter_context(tc.tile_pool(name="t4", bufs=1))
        tf_pool = ctx.enter_context(tc.tile_pool(name="tf", bufs=2))
        outp_pool = ctx.enter_context(tc.tile_pool(name="outp", bufs=1))

        a_sb = const_pool.tile([KDIM, NSH], dt.float16)
        nc.sync.dma_start(a_sb[:], a_dram[:])
        b_sb = const_pool.tile([KDIM, M], dt.float16)
        nc.sync.dma_start(b_sb[:, 0:GBLK], b_dram[:, 0:GBLK])
        nc.sync.dma_start(b_sb[:, GBLK:M], b_dram[:, GBLK:M])

        maccs = outp_pool.tile([128, NT], dt.float32)

        FBLK = 1024  # macc fold granularity
        nacc_prev = None
        tf = None
        for i in range(NT):
            q4 = i % 4
            if q4 == 0:
                # per-tile 256-wide row mins land here; one batched 1x reduce
                # every 4 tiles replaces four per-tile reduces
                tf = tf_pool.tile([128, 4, 256], dt.float16, tag="tf")
            last = i == NT - 1
            nacc_i = nacc_pool.tile([128, M], dt.float16, tag="nacc")
            if i == 0:
                c_i = nacc_i  # ACT converts straight into nacc_0
            elif i <= 3:
                c_i = c_pool.tile([128, M], dt.float16, tag="c")
            else:
                # steady tiles drain in PAIRS into one [128, 2, M] tile so the
                # tree L1 for both runs as a single batched 3D-AP instruction
                if i % 2 == 0:
                    c2 = c_pool.tile([128, 2, M], dt.float16, tag="c2")
                c_i = c2[:, i % 2, :]
            ma = None
            for g in range(M // GBLK):
                ps = psum_pool.tile([128, GBLK], dt.float32, tag="ps")
                for q in range(GBLK // MBLK):
                    j = g * (GBLK // MBLK) + q
                    nc.tensor.matmul(
                        ps[:, q * MBLK:(q + 1) * MBLK],
                        a_sb[0:KDIM, i * 128:(i + 1) * 128],
                        b_sb[0:KDIM, j * MBLK:(j + 1) * MBLK],
                        start=True,
                        stop=True,
                    )
                gs = slice(g * GBLK, (g + 1) * GBLK)
                nc.scalar.copy(c_i[:, gs], ps[:])

                if i <= 3:
                    # ramp-in rows (0-3): work incrementally per converted group so
                    # the DVE starts immediately while ACT builds its lead
                    if i > 0:
                        nc.vector.tensor_tensor(nacc_i[:, gs], c_i[:, gs], nacc_prev[:, gs], amin)
                    b0 = c_i[:, g * GBLK:g * GBLK + FBLK]
                    b1 = c_i[:, g * GBLK + FBLK:(g + 1) * GBLK]
                    if ma is None:
                        ma = macc_pool.tile([128, FBLK], dt.float16, tag="m0")
                        nc.vector.tensor_tensor(ma[:], b0, b1, amin)
                    else:
                        mb = macc_pool.tile([128, FBLK], dt.float16, tag="m0")
                        nc.vector.tensor_tensor(mb[:], ma[:], b0, amin)
                        mc = macc_pool.tile([128, FBLK], dt.float16, tag="m0")
                        nc.vector.tensor_tensor(mc[:], mb[:], b1, amin)
                        ma = mc

            if i > 3:
                # pair-batched tree L1 (on odd tiles, covering both pair rows):
                # it depends only on c2, so it can start as soon as the drains
                # land. Each pair's L1 fills two rows of the quad tile t4; the
                # deeper folds then run 4-tiles-at-a-time as single 3D-AP
                # instructions (same cycles, fewer per-instr init taxes).
                if q4 == 0:
                    t4 = t4_pool.tile([128, 4, M // 2], dt.float16, tag="t4")
                if i % 2 == 1:
                    nc.vector.tensor_tensor(
                        t4[:, q4 - 1:q4 + 1, :], c2[:, 0:2, 0:M // 2], c2[:, 0:2, M // 2:M], amin)
                if last:
                    # chunk the final nacc update so each slice's output DMA
                    # overlaps the remaining DVE work instead of serializing
                    # a full 2MB transfer after it
                    for ch in range(2):
                        cs = slice(ch * (M // 2), (ch + 1) * (M // 2))
                        nc.vector.tensor_tensor(nacc_i[:, cs], c_i[:, cs], nacc_prev[:, cs], amin)
                        nc.sync.dma_start(out_nacc[:, cs], nacc_i[:, cs])
                else:
                    nc.vector.tensor_tensor(nacc_i[:], c_i[:], nacc_prev[:], amin)
            nacc_prev = nacc_i
            if i <= 3:
                # finish ramp-in row's fold: 1024 -> 256 into tf
                nc.vector.tensor_tensor(ma[:, 0:512], ma[:, 0:512], ma[:, 512:1024], amin)
                nc.vector.tensor_tensor(tf[:, q4, :], ma[:, 0:256], ma[:, 256:512], amin)
            elif q4 == 3:
                # batched quad folds: 4096 -> 256 across all 4 rows at once
                w = M // 4
                while w >= 512:
                    nc.vector.tensor_tensor(t4[:, 0:4, 0:w], t4[:, 0:4, 0:w], t4[:, 0:4, w:2 * w], amin)
                    w //= 2
                nc.vector.tensor_tensor(tf[:, 0:4, :], t4[:, 0:4, 0:256], t4[:, 0:4, 256:512], amin)
            if q4 == 3:
                # two more batched folds 256 -> 64, then a single 1x reduce
                nc.vector.tensor_tensor(tf[:, 0:4, 0:128], tf[:, 0:4, 0:128], tf[:, 0:4, 128:256], amin)
                nc.vector.tensor_tensor(tf[:, 0:4, 0:64], tf[:, 0:4, 0:64], tf[:, 0:4, 64:128], amin)
                nc.vector.tensor_reduce(maccs[:, i - 3:i + 1], tf[:, 0:4, 0:64], axis=X, op=amin)

        nc.sync.dma_start(out_macc[:], maccs[:])

    nc.compile()
    return nc


def _get_nc():
    if "nc" not in _CACHE:
        _CACHE["nc"] = _build_bass()
    return _CACHE["nc"]


def _split16(v):
    hi = v.astype(np.float16)
    lo = (v - hi.astype(np.float32)).astype(np.float16)
    return hi, lo


def _make_in_maps(p, t):
    in_maps = []
    for c in range(NCORES):
        b, h = divmod(c, 2)
        ps = p[b, h * NSH:(h + 1) * NSH]        # (NSH, 3)
        pn = (ps.astype(np.float64) ** 2).sum(-1).astype(np.float32)
        tb = t[b]                               # (M, 3)
        tn = (tb.astype(np.float64) ** 2).sum(-1).astype(np.float32)

        A = np.empty((KDIM, NSH), np.float16)
        Bm = np.empty((KDIM, M), np.float16)
        # rows 4d..4d+3 per coord d: lhs [ah,ah,al,al] x rhs [th,tl,th,tl]
        for d in range(3):
            ah, al = _split16(-2.0 * ps[:, d])
            th, tl = _split16(tb[:, d])
            A[4 * d + 0] = ah
            A[4 * d + 1] = ah
            A[4 * d + 2] = al
            A[4 * d + 3] = al
            Bm[4 * d + 0] = th
            Bm[4 * d + 1] = tl
            Bm[4 * d + 2] = th
            Bm[4 * d + 3] = tl
        pnh, pnl = _split16(pn)
        tnh, tnl = _split16(tn)
        A[12] = pnh
        A[13] = pnl
        A[14] = 1.0
        A[15] = 1.0
        Bm[12] = 1.0
        Bm[13] = 1.0
        Bm[14] = tnh
        Bm[15] = tnl
        in_maps.append({"a": np.ascontiguousarray(A), "b": np.ascontiguousarray(Bm)})
    return in_maps


def _combine(results):
    total = 0.0
    for b in range(B):
        pred_sum = 0.0
        tmins = []
        for h in range(2):
            r = results[2 * b + h]
            macc = np.asarray(r["out_macc"], np.float64)        # (128, NT) d2
            pred_sum += np.sqrt(np.maximum(macc, 0.0)).sum()
            nacc = np.asarray(r["out_nacc"], np.float32)        # (128, M) d2
            tmins.append(nacc.min(axis=0))
        mean_pred = pred_sum / N
        d2t = np.maximum(np.minimum(tmins[0], tmins[1]), 0.0).astype(np.float64)
        mean_tgt = np.sqrt(d2t).mean()
        total += (mean_pred + mean_tgt) / 2.0
    return np.asarray(total / B, dtype=np.float32)


def run_on_cores(p, t, trace=False):
    """Run the bass kernel; returns (results, BassKernelResults)."""
    from concourse.bass_utils import run_bass_kernel_spmd

    nc = _get_nc()
    in_maps = _make_in_maps(p, t)
    br = run_bass_kernel_spmd(nc, in_maps, list(range(NCORES)), trace=trace)
    return br


def kernel(predicted_points, target_points):
    p = np.asarray(predicted_points, dtype=np.float32)
    t = np.asarray(target_points, dtype=np.float32)
    assert p.shape == (B, N, 3) and t.shape == (B, M, 3)
    br = run_on_cores(p, t, trace=False)
    return _combine(br.results)



# revision 25
# speedup vs baseline: 1.0063x; 1.0063x over previous
"""Chamfer distance kernel for 8 trn2 NeuronCores.

Sharding: data-parallel over batch B=4 (2 cores per batch element), with the
N=8192 predicted-point axis split in half across the core pair. Each core
computes, for its (batch, n-half):
  - d2[n, m] squared-distance tiles directly on the TensorEngine via a K=16
    matmul that folds the whole expression |p|^2 + |t|^2 - 2 p.t into one
    contraction: conceptual rows [-2*px, -2*py, -2*pz, 1, |p|^2] x
    [tx, ty, tz, |t|^2, 1], with every fp32 row split into an fp16 hi/lo pair
    (fp16 matmuls stream 1 col/cycle vs 4 for fp32; the hi/lo split keeps
    ~22 mantissa bits so d2 stays fp32-accurate). PSUM accumulates in fp32.
  - ScalarE converts PSUM fp32 -> SBUF fp16.
  - VectorE computes running elementwise min over n-tiles (-> per-m partial
    mins, [128, 8192], partition p holds min over n = i*128+p) and per-n mins
    over m (free-dim halving tree to 256; the last fold of each tile lands in
    a [128, 4, 256] staging tile so ONE 1x tensor_reduce covers 4 tiles).
Host: final partition-axis min, cross-core min, sqrt, means (fp64), scalar out.

DVE is the bottleneck (gapless at ~9.05us/tile = its busy time; ACT 80%, PE
36%, Pool unusable: walrus rejects TensorTensor on the GPSIMD engine and its
TensorCopy executes incorrectly via the PJRT path). Per-tile DVE floor =
nacc 8192 outs + tree ~8064 outs at 2 elem/cycle (2x_1p) ~= 8.9us; the tuning
left is edges only: tiles 0-3 process per-PSUM-group (DVE is ACT-starved
until ~tile 3), tree L1 is emitted before the nacc (fewer deps), and the last
tile's nacc is chunked so its output DMA overlaps remaining DVE work.
"""

import numpy as np

B = 4
N = 8192
M = 8192
NCORES = 8
NSH = N // 2          # predicted points per core
NT = NSH // 128       # 32 n-tiles per core
KDIM = 16             # fp16 hi/lo split rows (4 per coord + pn pair + tn pair)
MBLK = 512            # matmul free dim (one PSUM bank)
GBLK = 2048           # PSUM group (4 banks) converted per ACT op

_CACHE = {}


def _build_bass():
    from contextlib import ExitStack

    import concourse.bacc as bacc
    import concourse.mybir as mybir
    import concourse.tile as tile

    dt = mybir.dt
    amin = mybir.AluOpType.min
    X = mybir.AxisListType.X

    nc = bacc.Bacc(
        "TRN2",
        target_bir_lowering=False,
        debug=False,
        num_devices=NCORES,
    )
    a_dram = nc.declare_dram_parameter("a", [KDIM, NSH], dt.float16, isOutput=False)
    b_dram = nc.declare_dram_parameter("b", [KDIM, M], dt.float16, isOutput=False)
    out_macc = nc.declare_dram_parameter("out_macc", [128, NT], dt.float32, isOutput=True)
    out_nacc = nc.declare_dram_parameter("out_nacc", [128, M], dt.float16, isOutput=True)

    with ExitStack() as ctx:
        tc = ctx.enter_context(tile.TileContext(nc))
        const_pool = ctx.enter_context(tc.tile_pool(name="const", bufs=1))
        psum_pool = ctx.enter_context(tc.tile_pool(name="psum", bufs=2, space="PSUM"))
        c_pool = ctx.enter_context(tc.tile_pool(name="c", bufs=3))
        nacc_pool = ctx.enter_context(tc.tile_pool(name="nacc", bufs=2))
        macc_pool = ctx.enter_context(tc.tile_pool(name="macc", bufs=2))
        tf_pool = ctx.enter_context(tc.tile_pool(name="tf", bufs=2))
        outp_pool = ctx.enter_context(tc.tile_pool(name="outp", bufs=1))

        a_sb = const_pool.tile([KDIM, NSH], dt.float16)
        nc.sync.dma_start(a_sb[:], a_dram[:])
        b_sb = const_pool.tile([KDIM, M], dt.float16)
        nc.sync.dma_start(b_sb[:, 0:GBLK], b_dram[:, 0:GBLK])
        nc.sync.dma_start(b_sb[:, GBLK:M], b_dram[:, GBLK:M])

        maccs = outp_pool.tile([128, NT], dt.float32)

        FBLK = 1024  # macc fold granularity
        nacc_prev = None
        tf = None
        for i in range(NT):
            q4 = i % 4
            if q4 == 0:
                # per-tile 256-wide row mins land here; one batched 1x reduce
                # every 4 tiles replaces four per-tile reduces
                tf = tf_pool.tile([128, 4, 256], dt.float16, tag="tf")
            last = i == NT - 1
            nacc_i = nacc_pool.tile([128, M], dt.float16, tag="nacc")
            if i == 0:
                c_i = nacc_i  # ACT converts straight into nacc_0
            else:
                c_i = c_pool.tile([128, M], dt.float16, tag="c")
            ma = None
            for g in range(M // GBLK):
                ps = psum_pool.tile([128, GBLK], dt.float32, tag="ps")
                for q in range(GBLK // MBLK):
                    j = g * (GBLK // MBLK) + q
                    nc.tensor.matmul(
                        ps[:, q * MBLK:(q + 1) * MBLK],
                        a_sb[0:KDIM, i * 128:(i + 1) * 128],
                        b_sb[0:KDIM, j * MBLK:(j + 1) * MBLK],
                        start=True,
                        stop=True,
                    )
                gs = slice(g * GBLK, (g + 1) * GBLK)
                nc.scalar.copy(c_i[:, gs], ps[:])

                if i <= 3:
                    # ramp-in rows (0-3): work incrementally per converted group so
                    # the DVE starts immediately while ACT builds its lead
                    if i > 0:
                        nc.vector.tensor_tensor(nacc_i[:, gs], c_i[:, gs], nacc_prev[:, gs], amin)
                    b0 = c_i[:, g * GBLK:g * GBLK + FBLK]
                    b1 = c_i[:, g * GBLK + FBLK:(g + 1) * GBLK]
                    if ma is None:
                        ma = macc_pool.tile([128, FBLK], dt.float16, tag="m0")
                        nc.vector.tensor_tensor(ma[:], b0, b1, amin)
                    else:
                        mb = macc_pool.tile([128, FBLK], dt.float16, tag="m0")
                        nc.vector.tensor_tensor(mb[:], ma[:], b0, amin)
                        mc = macc_pool.tile([128, FBLK], dt.float16, tag="m0")
                        nc.vector.tensor_tensor(mc[:], mb[:], b1, amin)
                        ma = mc

            if i > 3:
                # tree L1 first: it depends only on c_i, so it can start as
                # soon as the drain lands (the nacc also needs nacc_prev).
                # Each tile's L1 lands in its row of the shared quad tile t4;
                # the deeper folds run 4-tiles-at-a-time as single 3D-AP
                # instructions (same cycles, 1/4 the per-instr init taxes).
                if q4 == 0:
                    t4 = macc_pool.tile([128, 4, M // 2], dt.float16, tag="ma")
                nc.vector.tensor_tensor(t4[:, q4, :], c_i[:, 0:M // 2], c_i[:, M // 2:M], amin)
                if last:
                    # chunk the final nacc update so each slice's output DMA
                    # overlaps the remaining DVE work instead of serializing
                    # a full 2MB transfer after it
                    for ch in range(2):
                        cs = slice(ch * (M // 2), (ch + 1) * (M // 2))
                        nc.vector.tensor_tensor(nacc_i[:, cs], c_i[:, cs], nacc_prev[:, cs], amin)
                        nc.sync.dma_start(out_nacc[:, cs], nacc_i[:, cs])
                else:
                    nc.vector.tensor_tensor(nacc_i[:], c_i[:], nacc_prev[:], amin)
            nacc_prev = nacc_i
            if i <= 3:
                # finish ramp-in row's fold: 1024 -> 256 into tf
                nc.vector.tensor_tensor(ma[:, 0:512], ma[:, 0:512], ma[:, 512:1024], amin)
                nc.vector.tensor_tensor(tf[:, q4, :], ma[:, 0:256], ma[:, 256:512], amin)
            elif q4 == 3:
                # batched quad folds: 4096 -> 256 across all 4 rows at once
                w = M // 4
                while w >= 512:
                    nc.vector.tensor_tensor(t4[:, 0:4, 0:w], t4[:, 0:4, 0:w], t4[:, 0:4, w:2 * w], amin)
                    w //= 2
                nc.vector.tensor_tensor(tf[:, 0:4, :], t4[:, 0:4, 0:256], t4[:, 0:4, 256:512], amin)
            if q4 == 3:
                # two more batched folds 256 -> 64, then a single 1x reduce
                nc.vector.tensor_tensor(tf[:, 0:4, 0:128], tf[:, 0:4, 0:128], tf[:, 0:4, 128:256], amin)
                nc.vector.tensor_tensor(tf[:, 0:4, 0:64], tf[:, 0:4, 0:64], tf[:, 0:4, 64:128], amin)
                nc.vector.tensor_reduce(maccs[:, i - 3:i + 1], tf[:, 0:4, 0:64], axis=X, op=amin)

        nc.sync.dma_start(out_macc[:], maccs[:])

    nc.compile()
    return nc


def _get_nc():
    if "nc" not in _CACHE:
        _CACHE["nc"] = _build_bass()
    return _CACHE["nc"]


def _split16(v):
    hi = v.astype(np.float16)
    lo = (v - hi.astype(np.float32)).astype(np.float16)
    return hi, lo


def _make_in_maps(p, t):
    in_maps = []
    for c in range(NCORES):
        b, h = divmod(c, 2)
        ps = p[b, h * NSH:(h + 1) * NSH]        # (NSH, 3)
        pn = (ps.astype(np.float64) ** 2).sum(-1).astype(np.float32)
        tb = t[b]                               # (M, 3)
        tn = (tb.astype(np.float64) ** 2).sum(-1).astype(np.float32)

        A = np.empty((KDIM, NSH), np.float16)
        Bm = np.empty((KDIM, M), np.float16)
        # rows 4d..4d+3 per coord d: lhs [ah,ah,al,al] x rhs [th,tl,th,tl]
        for d in range(3):
            ah, al = _split16(-2.0 * ps[:, d])
            th, tl = _split16(tb[:, d])
            A[4 * d + 0] = ah
            A[4 * d + 1] = ah
            A[4 * d + 2] = al
            A[4 * d + 3] = al
            Bm[4 * d + 0] = th
            Bm[4 * d + 1] = tl
            Bm[4 * d + 2] = th
            Bm[4 * d + 3] = tl
        pnh, pnl = _split16(pn)
        tnh, tnl = _split16(tn)
        A[12] = pnh
        A[13] = pnl
        A[14] = 1.0
        A[15] = 1.0
        Bm[12] = 1.0
        Bm[13] = 1.0
        Bm[14] = tnh
        Bm[15] = tnl
        in_maps.append({"a": np.ascontiguousarray(A), "b": np.ascontiguousarray(Bm)})
    return in_maps


def _combine(results):
    total = 0.0
    for b in range(B):
        pred_sum = 0.0
        tmins = []
        for h in range(2):
            r = results[2 * b + h]
            macc = np.asarray(r["out_macc"], np.float64)        # (128, NT) d2
            pred_sum += np.sqrt(np.maximum(macc, 0.0)).sum()
            nacc = np.asarray(r["out_nacc"], np.float32)        # (128, M) d2
            tmins.append(nacc.min(axis=0))
        mean_pred = pred_sum / N
        d2t = np.maximum(np.minimum(tmins[0], tmins[1]), 0.0).astype(np.float64)
        mean_tgt = np.sqrt(d2t).mean()
        total += (mean_pred + mean_tgt) / 2.0
    return np.asarray(total / B, dtype=np.float32)


def run_on_cores(p, t, trace=False):
    """Run the bass kernel; returns (results, BassKernelResults)."""
    from concourse.bass_utils import run_bass_kernel_spmd

    nc = _get_nc()
    in_maps = _make_in_maps(p, t)
    br = run_bass_kernel_spmd(nc, in_maps, list(range(NCORES)), trace=trace)
    return br


def kernel(predicted_points, target_points):
    p = np.asarray(predicted_points, dtype=np.float32)
    t = np.asarray(target_points, dtype=np.float32)
    assert p.shape == (B, N, 3) and t.shape == (B, M, 3)
    br = run_on_cores(p, t, trace=False)
    return _combine(br.results)

